# revision 6
# baseline (speedup 1.0000x reference)
"""Trainium2 Bass kernel: MultiHeadSelfAttention with RoPE.

Problem: B=4, T=2048, d_model=1024, 16 heads, d_head=64, fp32.
  Q = x@Wq.T; K = x@Wk.T; V = x@Wv.T  (zero biases; per-head RoPE on
  Q,K, interleaved even/odd pairs, freqs = arange(32)/10000)
  out = softmax(QK^T/8) @ V; y = out@Wo.T + bo

Sharding (8 cores): core c -> batch b=c//2, head-group g=c%2 (8 heads).
Each core computes its heads' attention over the full sequence and a
partial output projection (row-parallel out_proj); the host sums the two
partials per batch and adds bo.

v2 design (vs the v1 po_dram baseline):
  * Attention processes BOTH heads of a 128-row pair per step: the two
    K=64 score matmuls are packed into disjoint PE row-groups
    (tile_position (0,0)/(64,0)) so they stream concurrently, writing
    the two banks of one [128, 1024] PSUM tile; ONE ScalarE exp call
    covers both heads (halves per-call overhead).
  * tq is processed in 512-wide windows; ot0/ot1 accumulate exp(ST)^T V'
    (V carries a ones column per head -> row 64 is the softmax
    denominator).
  * po (normalized per-pair attention output, bf16) stays in SBUF; the
    out_proj consumes it directly as the stationary operand -- no DRAM
    round trip.
  * Cross-pair software pipelining at EMISSION level: the QKV/RoPE/V
    work of pair p+1 (and the out_proj for the last pair) is emitted in
    small chunks between attention groups of pair p, so the PE fills the
    gaps while ScalarE crunches exp.
  * bf16 for q/k tiles, po and wo (error contribution ~0.5% max);
    x/Wq/Wk/Wv/V stay float32r.
"""

import numpy as np

N_CORES = 8
B, T, D = 4, 2048, 1024
H, DH = 16, 64
THETA = 10000.0
P = 128
JW = 512          # per-core head-feature width (8 heads * 64)
DC = 8            # d_model / 128 contraction chunks
TW = T // 512     # 4 free-dim windows of 512 over t
PAIRS = JW // P   # 4 head-pairs per core
NTC = T // P      # 16 key chunks
EMIT_REPS = None   # test hook: loop the body on-device (timing only)

_CACHE = {}


def _round_f32r(a):
    """Round-half-even fp32 -> f32r (drop low 12 mantissa bits), matching
    the hardware cast."""
    ai = np.ascontiguousarray(a, np.float32).view(np.uint32).astype(np.uint64)
    lsb = (ai >> 12) & 1
    out = ((ai + 2047 + lsb) & 0xFFFFF000).astype(np.uint32)
    return out.view(np.float32)


def _build_program():
    import concourse.tile as tile
    from concourse import bacc, mybir

    f32 = mybir.dt.float32
    f32r = mybir.dt.float32r
    bf16 = mybir.dt.bfloat16
    nc = bacc.Bacc("TRN2", target_bir_lowering=False, debug=False,
                   num_devices=N_CORES)

    def inp(name, shape, dt=f32r):
        return nc.dram_tensor(name, shape, dt, kind="ExternalInput").ap()

    xt = inp("xt", [D, T])
    wq, wk, wv = inp("wq", [D, JW]), inp("wk", [D, JW]), inp("wv", [D, JW])
    wo = inp("wo", [JW, D], bf16)
    cos = inp("cos", [P, T], f32)
    sinswap = inp("sinswap", [P, T], f32)
    ident = inp("ident", [P, P], f32)
    vones = inp("vones", [P, NTC, 2])
    y = nc.dram_tensor("y", [T, D], f32, kind="ExternalOutput").ap()

    with tile.TileContext(nc) as tc:
        kw = dict(y=y, xt=xt, wq=wq, wk=wk, wv=wv,
                  wo=wo, cos=cos, sinswap=sinswap, ident=ident,
                  vones=vones)
        if EMIT_REPS:
            with tc.For_i(0, EMIT_REPS, 1):
                _emit(tc, nc, mybir, **kw)
        else:
            _emit(tc, nc, mybir, **kw)
    nc.compile()
    return nc


def _emit(tc, nc, mybir, *, y, xt, wq, wk, wv, wo, cos, sinswap,
          ident, vones):
    from contextlib import ExitStack

    f32 = mybir.dt.float32
    f32r = mybir.dt.float32r
    bf16 = mybir.dt.bfloat16
    Exp = mybir.ActivationFunctionType.Exp
    SWAP_MASK = [i ^ 1 for i in range(32)]

    with ExitStack() as ctx:
        static = ctx.enter_context(tc.tile_pool(name="static", bufs=1))

        xt_sb = static.tile([P, DC, T], f32r)
        cos_sb = static.tile([P, T], f32)
        sin_sb = static.tile([P, T], f32)
        ident_sb = static.tile([P, P], f32)
        wo_sb = static.tile([P, PAIRS, D], bf16)

        wpool = ctx.enter_context(tc.tile_pool(name="wpool", bufs=2))
        qkpool = ctx.enter_context(tc.tile_pool(name="qkpool", bufs=2))
        vpool = ctx.enter_context(tc.tile_pool(name="vpool", bufs=2))
        tmp = ctx.enter_context(tc.tile_pool(name="tmp", bufs=1))
        expp = ctx.enter_context(tc.tile_pool(name="expp", bufs=2))
        nrm = ctx.enter_context(tc.tile_pool(name="nrm", bufs=2))
        sopool = ctx.enter_context(tc.tile_pool(name="so", bufs=2))
        popool = ctx.enter_context(tc.tile_pool(name="po", bufs=1))
        ypool = ctx.enter_context(tc.tile_pool(name="ypool", bufs=2))
        mmps = ctx.enter_context(tc.tile_pool(name="mmps", bufs=2, space="PSUM"))
        stps = ctx.enter_context(tc.tile_pool(name="stps", bufs=2, space="PSUM"))
        otps = ctx.enter_context(tc.tile_pool(name="otps", bufs=1, space="PSUM"))

        # ---- initial loads: first QKV chain's operands first ----
        xt_re = xt.rearrange("(c p) t -> p c t", p=P)
        state = {}

        def load_pair_weights(p):
            jsl = slice(p * P, (p + 1) * P)
            w_sb = {}
            for name, ap in (("q", wq), ("k", wk), ("v", wv)):
                wt = wpool.tile([P, DC, P], f32r, tag=f"w_{name}",
                                name=f"w_{name}_{p}")
                nc.sync.dma_start(
                    wt[:], ap[:, jsl].rearrange("(c pp) j -> pp c j", pp=P))
                w_sb[name] = wt
            return w_sb

        jsl0 = slice(0, P)
        w0 = {}
        w0["q"] = wpool.tile([P, DC, P], f32r, tag="w_q", name="w_q_0")
        nc.sync.dma_start(
            w0["q"][:], wq[:, jsl0].rearrange("(c pp) j -> pp c j", pp=P))
        for dc in range(DC):
            nc.sync.dma_start(xt_sb[:, dc, :], xt_re[:, dc, :])
        for name, ap in (("k", wk), ("v", wv)):
            wt = wpool.tile([P, DC, P], f32r, tag=f"w_{name}",
                            name=f"w_{name}_0")
            nc.sync.dma_start(
                wt[:], ap[:, jsl0].rearrange("(c pp) j -> pp c j", pp=P))
            w0[name] = wt
        nc.sync.dma_start(cos_sb[:], cos[:])
        nc.sync.dma_start(sin_sb[:], sinswap[:])
        nc.sync.dma_start(ident_sb[:], ident[:])
        nc.sync.dma_start(wo_sb[:], wo.rearrange("(c p) m -> p c m", p=P))

        po_tiles = [popool.tile([P, T], bf16, tag=f"po{p}", name=f"po_{p}")
                    for p in range(PAIRS)]

        def pair_fill_gen(p, w_sb=None):
            """Emit QKV+RoPE+V for pair p in small chunks (one per yield)."""
            if w_sb is None:
                w_sb = load_pair_weights(p)
                yield
            v_sb = vpool.tile([P, NTC, 2, DH + 1], f32r, tag="v",
                              name=f"v_{p}")
            nc.sync.dma_start(v_sb[:, :, :, DH], vones[:])
            qk = {}
            for name in ("q", "k"):
                dst = qkpool.tile([P, T], bf16, tag=f"{name}t2",
                                  name=f"{name}t2_{p}")
                qk[name] = dst
                for tw in range(TW):
                    tsl = slice(tw * 512, (tw + 1) * 512)
                    ps = mmps.tile([P, 512], f32, tag="mm", name=f"qk_{p}")
                    for dc in range(DC):
                        nc.tensor.matmul(ps[:], lhsT=w_sb[name][:, dc, :],
                                         rhs=xt_sb[:, dc, tsl],
                                         start=(dc == 0), stop=(dc == DC - 1))
                        if dc == 3:
                            yield
                    yield
                    qs = tmp.tile([P, 512], f32, tag="ropetmp", name="qs")
                    nc.vector.tensor_mul(qs[:], ps[:], sin_sb[:, tsl])
                    nc.vector.tensor_mul(dst[:, tsl], ps[:], cos_sb[:, tsl])
                    qsw = tmp.tile([P, 512], f32, tag="ropesw", name="qsw")
                    nc.vector.stream_shuffle(qsw[:], qs[:], SWAP_MASK)
                    nc.vector.tensor_add(dst[:, tsl], dst[:, tsl], qsw[:])
                    yield
            for tw in range(TW):
                tsl = slice(tw * 512, (tw + 1) * 512)
                ps = mmps.tile([P, 512], f32, tag="mm", name=f"v_{p}")
                for dc in range(DC):
                    nc.tensor.matmul(ps[:], lhsT=w_sb["v"][:, dc, :],
                                     rhs=xt_sb[:, dc, tsl],
                                     start=(dc == 0), stop=(dc == DC - 1))
                    if dc == 3:
                        yield
                yield
                vt = tmp.tile([P, 512], f32, tag="vt", bufs=2, name="vt")
                nc.vector.tensor_copy(vt[:], ps[:])
                yield
                for i in range(4):
                    pv = mmps.tile([P, 512], f32, tag="mm", name=f"pv_{p}")
                    nc.tensor.transpose(pv[:, 0:P], vt[:, i * P:(i + 1) * P],
                                        ident_sb[:])
                    tci = tw * 4 + i
                    nc.vector.tensor_copy(
                        out=v_sb[:, tci, :, 0:DH],
                        in_=pv[:, 0:P].rearrange("t (g n) -> t g n", n=DH))
                    yield
            state[p] = (qk, v_sb)

        def outproj_window(w):
            """Emit out_proj for tq window w (one (tt, mw) chunk per yield)."""
            for tt in range(w * 4, (w + 1) * 4):
                tsl = slice(tt * P, (tt + 1) * P)
                for mw in range(D // 512):
                    msl = slice(mw * 512, (mw + 1) * 512)
                    ps = mmps.tile([P, 512], f32, tag="mm", name=f"op_{tt}")
                    for p in range(PAIRS):
                        nc.tensor.matmul(ps[:], lhsT=po_tiles[p][:, tsl],
                                         rhs=wo_sb[:, p, msl],
                                         start=(p == 0), stop=(p == PAIRS - 1))
                    yt = ypool.tile([P, 512], f32, tag="yt", name=f"yt_{tt}")
                    nc.vector.tensor_copy(yt[:], ps[:])
                    nc.sync.dma_start(y[tsl, msl], yt[:])
                    yield

        def outproj_gen():
            for w in range(TW):
                yield from outproj_window(w)

        def step(filler, n=1):
            for _ in range(n):
                if filler is None:
                    return
                try:
                    next(filler)
                except StopIteration:
                    return

        def drain(filler):
            if filler is not None:
                for _ in filler:
                    pass

        def attention_window(p, w, filler, budget):
            """ST+exp+AV over all key chunks for tq window w of pair p.
            Pulls up to `budget` chunks from `filler` between PE groups."""
            qk, v_sb = state[p]
            po_sb = po_tiles[p]
            tqsl = slice(w * 512, (w + 1) * 512)
            ot = [otps.tile([DH + 1, 512], f32, tag=f"ot{h}",
                            name=f"ot_{p}_{w}_{h}") for h in range(2)]

            def av(ex, tci):
                for h in range(2):
                    nc.tensor.matmul(ot[h][:], lhsT=v_sb[:, tci, h, :],
                                     rhs=ex[:, h * 512:(h + 1) * 512],
                                     start=(tci == 0), stop=(tci == NTC - 1))

            pend = None
            pulled = 0
            start_pull = NTC - budget if budget else NTC
            for tci in range(NTC):
                ksl = slice(tci * P, (tci + 1) * P)
                st = stps.tile([P, 1024], f32, tag="st", name=f"st_{p}_{w}")
                nc.tensor.matmul(st[:, 0:512], lhsT=qk["k"][0:DH, ksl],
                                 rhs=qk["q"][0:DH, tqsl],
                                 start=True, stop=True, tile_position=(0, 0))
                nc.tensor.matmul(st[:, 512:1024], lhsT=qk["k"][DH:P, ksl],
                                 rhs=qk["q"][DH:P, tqsl],
                                 start=True, stop=True, tile_position=(64, 0))
                if pend is not None:
                    av(*pend)
                ex = expp.tile([P, 1024], f32r, tag="exp", name=f"ex_{p}_{w}")
                nc.scalar.activation(ex[:], st[:], Exp, scale=0.125)
                pend = (ex, tci)
                if tci >= start_pull and pulled < budget:
                    step(filler)
                    pulled += 1
            av(*pend)
            # evacuate + normalize: row 64 of ot[h] is the denominator
            for h in range(2):
                so = sopool.tile([DH, 512], f32, tag=f"so{h}",
                                 name=f"so_{p}_{w}_{h}")
                s1 = nrm.tile([1, 512], f32, tag=f"s{h}", name=f"s_{p}_{w}_{h}")
                nc.vector.tensor_copy(so[:], ot[h][0:DH, :])
                nc.vector.tensor_copy(s1[:], ot[h][DH:DH + 1, :])
                rb = nrm.tile([DH, 512], f32, tag=f"rb{h}",
                              name=f"rb_{p}_{w}_{h}")
                nc.gpsimd.partition_broadcast(rb[:], s1[:])
                nc.vector.reciprocal(rb[:], rb[:])
                nc.vector.tensor_mul(po_sb[h * DH:(h + 1) * DH, tqsl],
                                     so[:], rb[:])

        # ---- main schedule ----
        drain(pair_fill_gen(0, w0))
        for p in range(PAIRS):
            if p < PAIRS - 1:
                filler = pair_fill_gen(p + 1)
                budgets = [16, 16, 16, 16]
            else:
                filler = outproj_gen()
                # window w's po is ready only after window w completes:
                # allow 8 out_proj chunks (window w-1's worth) per window
                budgets = [0, 8, 8, 8]
            for w in range(TW):
                attention_window(p, w, filler, budgets[w])
            drain(filler) if p < PAIRS - 1 else None
        drain(filler)  # tail: out_proj for the last window


def _rope_tables():
    # row r of a 128-row j-chunk: head-local index r%64, pair (r%64)//2
    r = np.arange(P)
    freqs = ((r % DH) // 2).astype(np.float32) * (1.0 / THETA)
    t = np.arange(T, dtype=np.float32)
    ang = t[None, :] * freqs[:, None]              # [128, T]
    cos = np.cos(ang).astype(np.float32)
    # sinswap[r] = sinpm[r^1]: +sin for even rows, -sin for odd rows
    sign = np.where(r % 2 == 0, 1.0, -1.0).astype(np.float32)
    sinswap = (np.sin(ang) * sign[:, None]).astype(np.float32)
    return cos, sinswap


def _host_inputs(x, Wq, Wk, Wv, Wo):
    import ml_dtypes

    cos, sinswap = _rope_tables()
    ident = np.eye(P, dtype=np.float32)
    vones = np.ones((P, NTC, 2), np.float32)
    wqT = _round_f32r(Wq.T)
    wkT = _round_f32r(Wk.T)
    wvT = _round_f32r(Wv.T)
    woT = Wo.T.astype(ml_dtypes.bfloat16)
    xtr = [_round_f32r(x[b].T) for b in range(B)]
    in_maps = []
    for c in range(N_CORES):
        b, g = divmod(c, 2)
        jsl = slice(g * JW, (g + 1) * JW)
        in_maps.append({
            "xt": xtr[b],
            "wq": np.ascontiguousarray(wqT[:, jsl]),
            "wk": np.ascontiguousarray(wkT[:, jsl]),
            "wv": np.ascontiguousarray(wvT[:, jsl]),
            "wo": np.ascontiguousarray(woT[jsl, :]),
            "cos": cos, "sinswap": sinswap, "ident": ident,
            "vones": vones,
        })
    return in_maps


def get_program():
    if "nc" not in _CACHE:
        _CACHE["nc"] = _build_program()
    return _CACHE["nc"]


def kernel(x, Wq, bq, Wk, bk, Wv, bv, Wo, bo):
    from concourse.bass_utils import run_bass_kernel_spmd

    x = np.asarray(x, np.float32)
    Wq, bq = np.asarray(Wq, np.float32), np.asarray(bq, np.float32)
    Wk, bk = np.asarray(Wk, np.float32), np.asarray(bk, np.float32)
    Wv, bv = np.asarray(Wv, np.float32), np.asarray(bv, np.float32)
    Wo, bo = np.asarray(Wo, np.float32), np.asarray(bo, np.float32)

    if np.any(bq) or np.any(bk) or np.any(bv):
        raise NotImplementedError(
            "nonzero qkv biases not supported (setup_inputs provides zeros)")
    nc = get_program()
    in_maps = _host_inputs(x, Wq, Wk, Wv, Wo)
    last_err = None
    for _attempt in range(3):
        try:
            res = run_bass_kernel_spmd(nc, in_maps, list(range(N_CORES)))
            break
        except Exception as e:  # transient device wedges; retry
            last_err = e
    else:
        raise last_err
    out = np.empty((B, T, D), np.float32)
    for b in range(B):
        out[b] = res.results[2 * b]["y"] + res.results[2 * b + 1]["y"] + bo
    return out
